# revision 1
# baseline (speedup 1.0000x reference)
"""Trainium2 Bass kernel for the disentangled non-local block.

Per batch b (one NeuronCore each, 8 batches over 8 cores):
  x:  [64, 4096]   (Cin x N, N = 64*64)
  q/k/v = 1x1 conv (64x64 GEMM + bias)
  q' = q - mean_n(q), k' = k - mean_n(k)
  pairwise: out_p[c,n] = sum_m softmax_m(q'_n . k'_m / 8) * v[c,m]
  unary:    out_u[c]   = sum_m softmax_m(q_mean . k'_m) * v[c,m]
  out = x + out_p + out_u

Layout strategy (per core):
  - q, k kept UNWHITENED: whitened logits differ from
    (q_n.k_m - qbar.k_m)/8 only by per-column-n terms that cancel in the
    softmax over m, so the whole whitening reduces to a per-partition
    bias -(qbar.k_m)/8 on the exp; the unary weights are
    softmax_m(qbar.k_m), i.e. exp of the same u_k vector.
  - S chunks computed with m on partitions: S[m(128), n(1024)] =
    matmul(lhsT=k[:, mchunk], rhs=q[:, nblock]).  exp on ScalarE
    (no max subtraction: logits are in [-4, 4] by construction).
  - v held transposed ([N, C] chunks) with a ones column appended, so
    the second matmul O[c(65), n] += vT_chunk.T @ E accumulates both
    the numerator (rows 0..63) and the softmax denominator (row 64).
  - denominator reciprocal (VectorE) broadcast across partitions via a
    K=1 ones-vector matmul; final combine on VectorE; residual add of
    (x + out_u) precomputed once; single contiguous 1 MB output DMA.

Matmul inputs are float32r (single-pass PE, 4x the fp32 rate) with the
E path in bf16; hardware-measured end-to-end relative error ~2e-4.
"""

import numpy as np

B = 8
CIN = 64
C = 64
H = W = 64
N = H * W            # 4096
NB = 1024            # n-block (columns per outer iteration)
NBLK = N // NB       # 4
MB = 128             # m-chunk (keys per matmul, partition dim)
MCH = N // MB        # 32
HALF = 512           # PSUM bank free-dim for fp32 matmul
SCALE = 0.125        # 1 / (sqrt(C) * temperature)

_CACHE = {}
BC_MODE = "mm"   # "mm" (PE matmul broadcast) or "dma" (step-0 DMA)
ABLATE = ""      # "", "noepi", "noacc", "sonly" (timing ablations)
RECIP = "dve"    # "dve" (VectorE iterative divide, off the ScalarE stream)\
                 # or "act" (exp(-ln D) inline on ScalarE)


def _build(repeat=1, compat=True):
    import concourse.bass as bass
    import concourse.tile as tile
    from concourse import mybir

    f32 = mybir.dt.float32
    AX = mybir.AxisListType
    AF = mybir.ActivationFunctionType

    # float32r: same 32-bit layout, single-pass PE matmul (4x faster than
    # float32 for moving dim >= 256) at reduced multiply precision.  The BIR
    # verifier requires every fp32r matmul input to be produced as fp32r
    # (rounded-on-write), so all matmul input tensors are declared float32r.
    f32r = mybir.dt.float32r
    bf16 = mybir.dt.bfloat16

    nc = bass.Bass()
    x_d = nc.dram_tensor("x_aug", [CIN + 1, N], f32r, kind="ExternalInput")
    wq_d = nc.dram_tensor("wqT_aug", [CIN + 1, 2 * C], f32r, kind="ExternalInput")
    wk_d = nc.dram_tensor("wkT_aug", [CIN + 1, 2 * C], f32r, kind="ExternalInput")
    wv_d = nc.dram_tensor("wvT_aug", [CIN + 1, C + 2], f32r, kind="ExternalInput")
    out_d = nc.dram_tensor("out", [C, N], f32, kind="ExternalOutput")

    with tile.TileContext(nc) as tc:
        with (
            tc.tile_pool(name="sing", bufs=1) as sing,
            tc.tile_pool(name="epool", bufs=4) as epool,
            tc.tile_pool(name="opool", bufs=2) as opool,
            tc.tile_pool(name="dpool", bufs=2) as dpool,
            tc.tile_pool(name="psA", bufs=2, space="PSUM") as psA,
            tc.tile_pool(name="psO", bufs=2, space="PSUM") as psO,
        ):
          for _rep in range(repeat):
            # ---- load inputs ----
            ones_sb = sing.tile([1, C], f32)
            nc.vector.memset(ones_sb, 1.0)
            wq_sb = sing.tile([CIN + 1, 2 * C], f32r)
            wk_sb = sing.tile([CIN + 1, 2 * C], f32r)
            wv_sb = sing.tile([CIN + 1, C + 2], f32r)
            nc.gpsimd.dma_start(out=wq_sb, in_=wq_d[:])
            nc.gpsimd.dma_start(out=wk_sb, in_=wk_d[:])
            nc.gpsimd.dma_start(out=wv_sb, in_=wv_d[:])
            x_sb = sing.tile([CIN + 1, N], f32r)
            for h in range(2):
                nc.sync.dma_start(
                    out=x_sb[:, h * (N // 2):(h + 1) * (N // 2)],
                    in_=x_d[:, h * (N // 2):(h + 1) * (N // 2)])

            # ---- q, k = W_aug @ x_aug  (bias via ones row of x_aug) ----
            # q, k stay UNWHITENED: the whitened logits differ from raw
            # q.k/8 - (qbar.k_m)/8 only by per-column-n terms, which cancel
            # in the softmax over m.  The per-m correction rides the exp's
            # per-partition bias operand.
            q_sb = sing.tile([C, N], f32r)
            k_sb = sing.tile([C, N], f32r)
            qsum8 = sing.tile([C, 8], f32)
            for j in range(N // HALF):
                qp = psA.tile([MB, HALF], f32, tag="S")
                nc.tensor.matmul(
                    qp, wq_sb, x_sb[:, j * HALF:(j + 1) * HALF],
                    start=True, stop=True,
                )
                nc.scalar.activation(
                    out=q_sb[:, j * HALF:(j + 1) * HALF], in_=qp[0:C, :],
                    func=AF.Copy, accum_out=qsum8[:, j:j + 1],
                )
                kp = psA.tile([MB, HALF], f32, tag="S")
                nc.tensor.matmul(
                    kp, wk_sb, x_sb[:, j * HALF:(j + 1) * HALF],
                    start=True, stop=True,
                )
                nc.vector.tensor_copy(
                    k_sb[:, j * HALF:(j + 1) * HALF], kp[0:C, :])

            # ---- u_k[m] = qbar . k_m  (exp bias; also the unary logits) ----
            qsum = sing.tile([C, 1], f32)
            nc.vector.reduce_sum(qsum, qsum8, axis=AX.X)
            qmean2 = sing.tile([C, 2], f32r)
            nc.vector.memset(qmean2.bitcast(f32), 0.0)
            nc.vector.tensor_scalar_mul(qmean2[:, 0:1], qsum, 1.0 / N)
            u_ps = psA.tile([MB, 2 * MCH], f32, tag="S")
            for t in range(MCH):
                nc.tensor.matmul(
                    u_ps[:, 2 * t:2 * t + 2], k_sb[:, t * MB:(t + 1) * MB],
                    qmean2, start=True, stop=True,
                )
            ubias = sing.tile([MB, MCH], f32)
            u_even = u_ps[:].rearrange("p (t two) -> p t two", two=2)[:, :, 0:1]
            nc.scalar.mul(ubias, u_even, -SCALE)
            eu = epool.tile([MB, 2 * MCH], bf16, tag="E")
            nc.scalar.activation(out=eu, in_=u_ps, func=AF.Exp)

            # ---- vT chunks ([m, c] layout) with ones column ----
            # (emitted after whiten: keeps the DVE queue clear for the
            # whiten ops that gate the main loop)
            vT_sb = sing.tile([MB, MCH, MB], bf16)
            nc.vector.memset(vT_sb, 0.0)
            for t in range(MCH):
                vp = psO.tile([MB, C + 2], f32, tag="O")
                nc.tensor.matmul(
                    vp, x_sb[:, t * MB:(t + 1) * MB], wv_sb,
                    start=True, stop=True,
                )
                nc.vector.tensor_copy(vT_sb[:, t, 0:C + 1], vp[:, 0:C + 1])

            def emit_unary():
                # unary attention: weights are softmax_m(u_k[m]) (constant
                # offsets cancel), so eu = exp(u_ps) computed above feeds
                # the accumulation directly
                uacc = psO.tile([MB, 2], f32, tag="O")
                for t in range(MCH):
                    nc.tensor.matmul(
                        uacc, vT_sb[:, t, :], eu[:, 2 * t:2 * t + 2],
                        start=(t == 0), stop=(t == MCH - 1),
                    )
                du = sing.tile([1, 1], f32)
                nc.vector.tensor_copy(du, uacc[C:C + 1, 0:1])
                recu = sing.tile([1, 1], f32)
                nc.vector.reciprocal(recu, du)
                if BC_MODE == "dma":
                    bcu = sing.tile([C, 1], f32)
                    rau = recu[:]
                    nc.sync.dma_start(out=bcu, in_=bass.AP(
                        tensor=rau.tensor, offset=rau.offset,
                        ap=[rau.ap[0], [0, C], [1, 1]]))
                else:
                    bcu_ps = psO.tile([C, 1], f32, tag="O")
                    nc.tensor.matmul(bcu_ps, ones_sb, recu,
                                     start=True, stop=True)
                    bcu = sing.tile([C, 1], f32)
                    nc.vector.tensor_copy(bcu, bcu_ps)
                ucp = sing.tile([C, 1], f32)
                nc.vector.tensor_copy(ucp, uacc[0:C, 0:1])
                u_sb = sing.tile([C, 1], f32)
                nc.vector.tensor_mul(u_sb, ucp, bcu)
                # x + unary term, broadcast along n
                xpu = sing.tile([C, N], f32)
                nc.vector.tensor_scalar_add(xpu, x_sb[0:C, :], u_sb)
                return xpu

            # ---- main attention loop ----
            # each block's epilogue is emitted two exps INTO the next block:
            # its ln/exp-recip ScalarE ops depend on the block's last acc
            # matmul, and emitting them at the block boundary would stall
            # ScalarE's strict FIFO for ~3us per block
            # each block's epilogue is deferred two tiles into the next
            # block: its Ln/Exp ops depend on the block's last acc matmul
            # and would stall ScalarE's strict FIFO ~3us at every block
            # boundary if emitted inline.  The 1/D broadcast goes into the
            # accumulator's own unused rows 64-127 (tile_position col 64),
            # so the deferred epilogue allocates no PSUM and cannot create
            # a pool-slot cycle.
            out_sb = None if ABLATE else sing.tile([C, N], f32)

            def make_epilogue(j, o_ps):
                def epi():
                    rec = dpool.tile([1, NB], f32, tag="rec")
                    recip = "act" if j == NBLK - 1 else RECIP
                    if recip == "act":
                        dln = dpool.tile([1, NB], f32, tag="d")
                        nc.scalar.activation(out=dln, in_=o_ps[C:C + 1, :],
                                             func=AF.Ln)
                        nc.scalar.activation(out=rec, in_=dln, func=AF.Exp,
                                             scale=-1.0)
                    else:
                        d_sb = dpool.tile([1, NB], f32, tag="d")
                        nc.vector.tensor_copy(d_sb, o_ps[C:C + 1, :])
                        nc.vector.reciprocal(rec, d_sb)
                    for h in range(NB // HALF):
                        nc.tensor.matmul(
                            o_ps[C:2 * C, h * HALF:(h + 1) * HALF], ones_sb,
                            rec[:, h * HALF:(h + 1) * HALF],
                            start=True, stop=True, tile_position=(0, C),
                        )
                    bcs = opool.tile([C, NB], f32, tag="bcs")
                    nc.vector.tensor_copy(bcs, o_ps[C:2 * C, :])
                    o_sb = out_sb[:, j * NB:(j + 1) * NB]
                    nc.vector.tensor_mul(o_sb, o_ps[0:C, :], bcs)
                    nc.vector.tensor_add(o_sb, o_sb,
                                         xpu[:, j * NB:(j + 1) * NB])
                return epi

            pending_epi = None
            for j in range(NBLK):
                o_ps = psO.tile([MB, NB], f32, tag="O")
                for t in range(MCH):
                    if t == 2 and pending_epi is not None:
                        pending_epi()
                        pending_epi = None
                    s_ps = psA.tile([MB, NB], f32, tag="S")
                    for h in range(NB // HALF):
                        nc.tensor.matmul(
                            s_ps[:, h * HALF:(h + 1) * HALF],
                            k_sb[:, t * MB:(t + 1) * MB],
                            q_sb[:, j * NB + h * HALF:j * NB + (h + 1) * HALF],
                            start=True, stop=True,
                        )
                    if ABLATE == "sonly":
                        continue
                    e_sb = epool.tile([MB, NB], bf16, tag="E")
                    nc.scalar.activation(out=e_sb, in_=s_ps, func=AF.Exp,
                                         scale=SCALE,
                                         bias=ubias[:, t:t + 1])
                    if ABLATE == "noacc":
                        continue
                    for h in range(NB // HALF):
                        nc.tensor.matmul(
                            o_ps[:, h * HALF:(h + 1) * HALF],
                            vT_sb[:, t, :],
                            e_sb[:, h * HALF:(h + 1) * HALF],
                            start=(t == 0), stop=(t == MCH - 1),
                        )
                if j == 0:
                    xpu = emit_unary()
                if ABLATE:
                    continue
                pending_epi = make_epilogue(j, o_ps)
            if pending_epi is not None:
                pending_epi()
            if not ABLATE:
                nc.sync.dma_start(out=out_d[:], in_=out_sb)

    if compat:
        _fix_walrus_compat(nc)
    return nc


def _fix_walrus_compat(nc):
    """Work around version skew between concourse and this walrus build.

    1. This walrus accepts at most ONE sync wait per instruction
       (setupSyncWait: "Too many sync wait commands").  Excess waits move
       to same-engine NOPs inserted immediately before the instruction —
       engine program order preserves the wait-before-execute semantics.
    2. EVENT_SEMAPHORE_RANGE_CLEAR (emitted by TileContext exit to reset
       tile semaphores) has a different ISA struct length in this walrus
       ("ISA wrong length").  Replace with one NOP per semaphore carrying
       a sem-wr-imm 0 update.
    """
    from concourse import mybir

    for f in nc.m.functions:
        for blk in f.blocks:
            new = []
            for inst in blk.instructions:
                si = inst.sync_info
                if (type(inst).__name__ == "InstISA"
                        and getattr(inst, "op_name", None)
                        == "EVENT_SEMAPHORE_RANGE_CLEAR"):
                    d = inst.ant_dict
                    first, last = d["range_first"], d["range_last"]
                    waits = list(si.on_wait) if si else []
                    for s in range(first, last + 1):
                        upd = mybir.SyncUpdate(
                            sync_type="semaphore", id=s,
                            ant_name=f"semreset_{s}",
                            update_mode="sem-wr-imm", update_value=0,
                            update_reg=None)
                        nop = mybir.InstNoOp(
                            name=f"semreset_{nc.next_id()}",
                            sync_info=mybir.SyncInfo(
                                on_wait=[waits.pop()] if waits else [],
                                on_update=[upd]),
                            bass_nofuse=True,
                            engine=inst.engine)
                        new.append(nop)
                    while waits:
                        nop = mybir.InstNoOp(
                            name=f"semreset_{nc.next_id()}",
                            sync_info=mybir.SyncInfo(
                                on_wait=[waits.pop()], on_update=[]),
                            bass_nofuse=True, engine=inst.engine)
                        new.insert(0, nop)
                    continue
                if si is not None and len(si.on_wait) > 1:
                    waits = list(si.on_wait)
                    excess, keep = waits[:-1], waits[-1:]
                    for w in excess:
                        nop = mybir.InstNoOp(
                            name=f"mwfix_{nc.next_id()}",
                            sync_info=mybir.SyncInfo(on_wait=[w], on_update=[]),
                            bass_nofuse=True,
                            engine=inst.engine)
                        new.append(nop)
                    inst.sync_info = mybir.SyncInfo(
                        on_wait=keep, on_update=list(si.on_update))
                new.append(inst)
            blk.instructions[:] = new


def _prep_inputs(x, wq, bq, wk, bk, wv, bv):
    """Host-side shard prep: per-core input maps (batch i -> core i)."""
    x = np.asarray(x, np.float32)
    # weights padded to 128 output columns (fp32r matmul needs col_grp
    # 0xf => stationary free dim 128); wv padded to 66 (even moving dim)
    wqT = np.zeros((CIN + 1, 2 * C), np.float32)
    wqT[:CIN, :C] = np.asarray(wq, np.float32).T
    wqT[CIN, :C] = np.asarray(bq, np.float32)
    wkT = np.zeros((CIN + 1, 2 * C), np.float32)
    wkT[:CIN, :C] = np.asarray(wk, np.float32).T
    wkT[CIN, :C] = np.asarray(bk, np.float32)
    wvT = np.zeros((CIN + 1, C + 2), np.float32)
    wvT[:CIN, :C] = np.asarray(wv, np.float32).T
    wvT[CIN, :C] = np.asarray(bv, np.float32)
    wvT[CIN, C] = 1.0
    ones = np.ones((1, N), np.float32)
    maps = []
    for i in range(B):
        xa = np.concatenate([x[i].reshape(CIN, N), ones], 0)
        maps.append({"x_aug": np.ascontiguousarray(xa),
                     "wqT_aug": wqT, "wkT_aug": wkT, "wvT_aug": wvT})
    return maps


def kernel(x, wq, bq, wk, bk, wv, bv):
    from concourse.bass_utils import run_bass_kernel_spmd

    if "nc" not in _CACHE:
        _CACHE["nc"] = _build()
    nc = _CACHE["nc"]
    in_maps = _prep_inputs(x, wq, bq, wk, bk, wv, bv)
    res = run_bass_kernel_spmd(nc, in_maps, list(range(B)))
    out = np.stack([res.results[i]["out"].reshape(C, H, W) for i in range(B)])
    return out.astype(np.float32)



# revision 17
# speedup vs baseline: 1.0549x; 1.0549x over previous
"""Trainium2 Bass kernel for the disentangled non-local block.

Per batch b (one NeuronCore each, 8 batches over 8 cores):
  x:  [64, 4096]   (Cin x N, N = 64*64)
  q/k/v = 1x1 conv (64x64 GEMM + bias)
  q' = q - mean_n(q), k' = k - mean_n(k)
  pairwise: out_p[c,n] = sum_m softmax_m(q'_n . k'_m / 8) * v[c,m]
  unary:    out_u[c]   = sum_m softmax_m(q_mean . k'_m) * v[c,m]
  out = x + out_p + out_u

Layout strategy (per core):
  - q, k kept UNWHITENED: whitened logits differ from raw q.k/8 only by
    per-column-n terms that cancel in the softmax over m; the per-m
    correction -(qbar.k_m)/8 rides the exp's per-partition bias operand.
  - S chunks with m on partitions: S[m(128), n(1024)] via TWO concurrent
    row-tiled matmuls (contraction K=64 uses half the PE rows, so the
    two 512-column halves of each chunk run in row groups (0,0)/(64,0)
    against duplicated copies of k/q held in partitions 0-63 and
    64-127; the duplication is free - the qk GEMM weights are stored
    twice so the GEMM emits both copies).
  - exp is split across TWO engines: ScalarE runs the real exp
    (scale/bias operands fold the whitening), and VectorE handles a
    subset of chunks via the Schraudolph bitcast trick: with
    wk pre-scaled by 2^7/ln2 * (1/8) on the host, bf16(exp(y)) ~=
    bitcast_bf16(int16(s' + bexp[m])), one fused tensor_scalar op per
    chunk (add per-partition bexp, int16-convert-on-write into the
    bf16 e tile).  Softmax renormalization cancels the ~3% sawtooth.
  - v held transposed with a ones column so the acc matmul
    O[c(65), n] += vT.T @ E accumulates numerator + denominator.
  - denominator reciprocal broadcast across partitions via a K=1 ones
    matmul into the accumulator's unused rows 64-127; combine on DVE;
    per-block output DMA overlaps the next block's compute.

Matmul inputs are float32r (single-pass PE) with the E path in bf16.
"""

import math
import numpy as np

B = 8
CIN = 64
C = 64
H = W = 64
N = H * W            # 4096
NB = 1024            # n-block (columns per outer iteration)
NBLK = N // NB       # 4
MB = 128             # m-chunk (keys per matmul, partition dim)
MCH = N // MB        # 32
HALF = 512           # PSUM bank free-dim for fp32 matmul
SCALE = 0.125        # 1 / (sqrt(C) * temperature)

APRIME = 8.0 / math.log(2.0)        # fp8e4m3-bit-domain exp slope
CSH = 0.35                          # Schraudolph minimax shift
BEXP0 = 7.0 * 8.0 - CSH + 0.5       # +0.5: int8 convert truncates
KSCL = APRIME * SCALE               # folded into wk/bk on the host

DVE_T = 12           # m-chunks per 32 handled by VectorE Schraudolph
DVE_SET = {t for t in range(MCH)
           if (t + 1) * DVE_T // MCH > t * DVE_T // MCH}
VPAD = 80            # fp8 vT pair stride (DoubleRow needs step%16==0)

_CACHE = {}


def _build(repeat=1, compat=True):
    import concourse.bass as bass
    import concourse.tile as tile
    from concourse import mybir
    from concourse.alu_op_type import AluOpType

    f32 = mybir.dt.float32
    i8 = mybir.dt.int8
    fp8 = mybir.dt.float8e4
    AX = mybir.AxisListType
    AF = mybir.ActivationFunctionType
    DR = mybir.MatmulPerfMode.DoubleRow

    f32r = mybir.dt.float32r
    bf16 = mybir.dt.bfloat16

    nc = bass.Bass()
    x_d = nc.dram_tensor("x_aug", [CIN + 1, N], f32r, kind="ExternalInput")
    wq_d = nc.dram_tensor("wqT_aug", [CIN + 1, 2 * C], f32r, kind="ExternalInput")
    wk_d = nc.dram_tensor("wkT_aug", [CIN + 1, 2 * C], f32r, kind="ExternalInput")
    wv_d = nc.dram_tensor("wvT_aug", [CIN + 1, C + 2], f32r, kind="ExternalInput")
    out_d = nc.dram_tensor("out", [C, N], f32, kind="ExternalOutput")

    with tile.TileContext(nc) as tc:
        with (
            tc.tile_pool(name="sing", bufs=1) as sing,
            tc.tile_pool(name="epool", bufs=4) as epool,
            tc.tile_pool(name="dpool", bufs=2) as dpool,
            tc.tile_pool(name="psA", bufs=3, space="PSUM") as psA,
            tc.tile_pool(name="psO", bufs=1, space="PSUM") as psO,
        ):
          for _rep in range(repeat):
            # ---- load inputs ----
            ones_sb = sing.tile([1, C], f32)
            nc.vector.memset(ones_sb, 1.0)
            wq_sb = sing.tile([CIN + 1, 2 * C], f32r)
            wk_sb = sing.tile([CIN + 1, 2 * C], f32r)
            wv_sb = sing.tile([CIN + 1, C + 2], f32r)
            nc.gpsimd.dma_start(out=wq_sb, in_=wq_d[:])
            nc.gpsimd.dma_start(out=wk_sb, in_=wk_d[:])
            nc.gpsimd.dma_start(out=wv_sb, in_=wv_d[:])
            x_sb = sing.tile([CIN + 1, N], f32r)
            for h, eng in enumerate((nc.sync, nc.scalar, nc.sync,
                                     nc.gpsimd)):
                eng.dma_start(
                    out=x_sb[:, h * (N // 4):(h + 1) * (N // 4)],
                    in_=x_d[:, h * (N // 4):(h + 1) * (N // 4)])

            # ---- q2, k2 = W_aug @ x_aug, duplicated to 128 partitions ----
            # (wq/wk stationary columns hold each output channel twice, so
            # the GEMM lands q and k in partitions 0-63 AND 64-127; the
            # copies feed the two S row-tiles.)  wk/bk are pre-scaled by
            # KSCL on the host, so s' = KSCL * (k.q) directly.
            q2_sb = sing.tile([2 * C, N], f32r)
            k2_sb = sing.tile([2 * C, N], f32r)
            qsum8 = sing.tile([2 * C, 8], f32)
            for j in range(N // HALF):
                qp = psA.tile([MB, HALF], f32, tag="S")
                nc.tensor.matmul(
                    qp, wq_sb, x_sb[:, j * HALF:(j + 1) * HALF],
                    start=True, stop=True,
                )
                nc.scalar.activation(
                    out=q2_sb[:, j * HALF:(j + 1) * HALF], in_=qp,
                    func=AF.Copy, accum_out=qsum8[:, j:j + 1],
                )
                kp = psA.tile([MB, HALF], f32, tag="S")
                nc.tensor.matmul(
                    kp, wk_sb, x_sb[:, j * HALF:(j + 1) * HALF],
                    start=True, stop=True,
                )
                nc.vector.tensor_copy(
                    k2_sb[:, j * HALF:(j + 1) * HALF], kp)

            # ---- u'_m = qbar . k'_m  (scaled-k units: u' = KSCL*qbar.k_m) --
            qsum = sing.tile([2 * C, 1], f32)
            nc.vector.reduce_sum(qsum, qsum8, axis=AX.X)
            qmean2 = sing.tile([2 * C, 2], f32r)
            nc.vector.memset(qmean2.bitcast(f32), 0.0)
            nc.vector.tensor_scalar_mul(qmean2[:, 0:1], qsum, 1.0 / N)
            u_ps = psA.tile([MB, 2 * MCH], f32, tag="S")
            for t in range(MCH):
                nc.tensor.matmul(
                    u_ps[:, 2 * t:2 * t + 2],
                    k2_sb[0:C, t * MB:(t + 1) * MB],
                    qmean2[0:C, :], start=True, stop=True,
                )
            u_even = u_ps[:].rearrange("p (t two) -> p t two", two=2)[:, :, 0:1]
            # ScalarE chunk bias: exp arg = s'/APRIME + ubias, ubias = -u'/A'
            ubias = sing.tile([MB, MCH], f32)
            nc.scalar.mul(ubias, u_even, -1.0 / APRIME)
            # VectorE chunk bias: int16 bits = s' + (BEXP0 - u')
            bexp = sing.tile([MB, MCH], f32)
            nc.vector.tensor_scalar(
                out=bexp, in0=u_even, scalar1=-1.0, scalar2=BEXP0,
                op0=AluOpType.mult, op1=AluOpType.add)
            # unary weights: softmax_m(qbar.k_m) (original scale: u'/KSCL)
            eu = epool.tile([MB, 2 * MCH], bf16, tag="E")
            nc.scalar.activation(out=eu, in_=u_ps, func=AF.Exp,
                                 scale=1.0 / KSCL)

            # ---- vT chunks ([m, c] layout, fp8e4m3) with ones column ----
            # pairs of chunks stored as [pair, sub, c] for the DoubleRow
            # acc matmul (virtual K=256: two m-chunks per pass)
            vT_sb = sing.tile([MB, MCH // 2, 2, VPAD], fp8)
            nc.vector.memset(vT_sb, 0.0)
            for t in range(MCH):
                vp = psA.tile([MB, C + 2], f32, tag="S")
                nc.tensor.matmul(
                    vp, x_sb[:, t * MB:(t + 1) * MB], wv_sb,
                    start=True, stop=True,
                )
                nc.vector.tensor_copy(vT_sb[:, t // 2, t % 2, 0:C + 1],
                                      vp[:, 0:C + 1])

            def emit_unary():
                uacc = psA.tile([MB, 2], f32, tag="S")
                for t in range(MCH):
                    nc.tensor.matmul(
                        uacc[0:VPAD, :], vT_sb[:, t // 2, t % 2, :],
                        eu[:, 2 * t:2 * t + 2],
                        start=(t == 0), stop=(t == MCH - 1),
                    )
                du = sing.tile([1, 1], f32)
                nc.vector.tensor_copy(du, uacc[C:C + 1, 0:1])
                recu = sing.tile([1, 1], f32)
                nc.vector.reciprocal(recu, du)
                bcu_ps = psA.tile([C, 1], f32, tag="S")
                nc.tensor.matmul(bcu_ps, ones_sb, recu, start=True, stop=True)
                bcu = sing.tile([C, 1], f32)
                nc.vector.tensor_copy(bcu, bcu_ps)
                ucp = sing.tile([C, 1], f32)
                nc.vector.tensor_copy(ucp, uacc[0:C, 0:1])
                u_sb = sing.tile([C, 1], f32)
                nc.vector.tensor_mul(u_sb, ucp, bcu)
                # x + unary term, broadcast along n
                xpu = sing.tile([C, N], f32)
                nc.vector.tensor_scalar_add(xpu, x_sb[0:C, :], u_sb)
                return xpu

            # unary term + residual, before the main loop (keeps psO
            # exclusively for the block accumulator)
            xpu = emit_unary()

            # ---- main attention loop ----
            out_sb = sing.tile([C, N], f32)

            def epilogue(j, o_ps):
                    rec = dpool.tile([1, NB], f32, tag="rec")
                    if j == NBLK - 1:
                        dln = dpool.tile([1, NB], f32, tag="d")
                        nc.scalar.activation(out=dln, in_=o_ps[C:C + 1, :],
                                             func=AF.Ln)
                        nc.scalar.activation(out=rec, in_=dln, func=AF.Exp,
                                             scale=-1.0)
                    else:
                        d_sb = dpool.tile([1, NB], f32, tag="d")
                        nc.vector.tensor_copy(d_sb, o_ps[C:C + 1, :])
                        nc.vector.reciprocal(rec, d_sb)
                    for h in range(NB // HALF):
                        nc.tensor.matmul(
                            o_ps[C:2 * C, h * HALF:(h + 1) * HALF], ones_sb,
                            rec[:, h * HALF:(h + 1) * HALF],
                            start=True, stop=True, tile_position=(0, C),
                        )
                    # (a DVE op may read only ONE non-scalar input from
                    # PSUM, so the broadcast goes through SBUF)
                    bcs = dpool.tile([C, NB], f32, tag="bcs")
                    nc.vector.tensor_copy(bcs, o_ps[C:2 * C, :])
                    o_sb = out_sb[:, j * NB:(j + 1) * NB]
                    nc.vector.tensor_mul(o_sb, o_ps[0:C, :], bcs)
                    nc.vector.tensor_add(o_sb, o_sb,
                                         xpu[:, j * NB:(j + 1) * NB])
                    nc.sync.dma_start(out=out_d[:, j * NB:(j + 1) * NB],
                                      in_=o_sb)

            for j in range(NBLK):
                o_ps = psO.tile([MB, NB], f32, tag="O")
                for u in range(MCH // 2):
                    e8 = epool.tile([MB, 2, NB], fp8, tag="E")
                    for s in range(2):
                        t = 2 * u + s
                        s_ps = psA.tile([MB, NB], f32, tag="S")
                        # two concurrent row-tiled matmuls: chunk t's
                        # n-halves run in PE row groups 0-63 / 64-127
                        nc.tensor.matmul(
                            s_ps[:, 0:HALF],
                            k2_sb[0:C, t * MB:(t + 1) * MB],
                            q2_sb[0:C, j * NB:j * NB + HALF],
                            start=True, stop=True,
                        )
                        nc.tensor.matmul(
                            s_ps[:, HALF:NB],
                            k2_sb[C:2 * C, t * MB:(t + 1) * MB],
                            q2_sb[C:2 * C, j * NB + HALF:(j + 1) * NB],
                            start=True, stop=True,
                        )
                        if t in DVE_SET:
                            nc.vector.tensor_scalar(
                                out=e8[:, s, :].bitcast(i8), in0=s_ps,
                                scalar1=bexp[:, t:t + 1], scalar2=0.5,
                                op0=AluOpType.add, op1=AluOpType.max)
                        else:
                            nc.scalar.activation(
                                out=e8[:, s, :], in_=s_ps, func=AF.Exp,
                                scale=1.0 / APRIME, bias=ubias[:, t:t + 1])
                    # DoubleRow: both chunks of the pair in one pass
                    for h in range(NB // HALF):
                        nc.tensor.matmul(
                            o_ps[0:VPAD, h * HALF:(h + 1) * HALF],
                            vT_sb[:, u, :, :],
                            e8[:, :, h * HALF:(h + 1) * HALF],
                            start=(u == 0), stop=(u == MCH // 2 - 1),
                            perf_mode=DR,
                        )
                epilogue(j, o_ps)

    if compat:
        _fix_walrus_compat(nc)
    return nc


def _fix_walrus_compat(nc):
    """Work around version skew between concourse and this walrus build.

    1. This walrus accepts at most ONE sync wait per instruction
       (setupSyncWait: "Too many sync wait commands").  Excess waits move
       to same-engine NOPs inserted immediately before the instruction —
       engine program order preserves the wait-before-execute semantics.
    2. EVENT_SEMAPHORE_RANGE_CLEAR (emitted by TileContext exit to reset
       tile semaphores) has a different ISA struct length in this walrus
       ("ISA wrong length").  Replace with one NOP per semaphore carrying
       a sem-wr-imm 0 update.
    """
    from concourse import mybir

    for f in nc.m.functions:
        for blk in f.blocks:
            new = []
            for inst in blk.instructions:
                si = inst.sync_info
                if (type(inst).__name__ == "InstISA"
                        and getattr(inst, "op_name", None)
                        == "EVENT_SEMAPHORE_RANGE_CLEAR"):
                    d = inst.ant_dict
                    first, last = d["range_first"], d["range_last"]
                    waits = list(si.on_wait) if si else []
                    for s in range(first, last + 1):
                        upd = mybir.SyncUpdate(
                            sync_type="semaphore", id=s,
                            ant_name=f"semreset_{s}",
                            update_mode="sem-wr-imm", update_value=0,
                            update_reg=None)
                        nop = mybir.InstNoOp(
                            name=f"semreset_{nc.next_id()}",
                            sync_info=mybir.SyncInfo(
                                on_wait=[waits.pop()] if waits else [],
                                on_update=[upd]),
                            bass_nofuse=True,
                            engine=inst.engine)
                        new.append(nop)
                    while waits:
                        nop = mybir.InstNoOp(
                            name=f"semreset_{nc.next_id()}",
                            sync_info=mybir.SyncInfo(
                                on_wait=[waits.pop()], on_update=[]),
                            bass_nofuse=True, engine=inst.engine)
                        new.insert(0, nop)
                    continue
                if si is not None and len(si.on_wait) > 1:
                    waits = list(si.on_wait)
                    excess, keep = waits[:-1], waits[-1:]
                    for w in excess:
                        nop = mybir.InstNoOp(
                            name=f"mwfix_{nc.next_id()}",
                            sync_info=mybir.SyncInfo(on_wait=[w], on_update=[]),
                            bass_nofuse=True,
                            engine=inst.engine)
                        new.append(nop)
                    inst.sync_info = mybir.SyncInfo(
                        on_wait=keep, on_update=list(si.on_update))
                new.append(inst)
            blk.instructions[:] = new


def _prep_inputs(x, wq, bq, wk, bk, wv, bv):
    """Host-side shard prep: per-core input maps (batch i -> core i)."""
    x = np.asarray(x, np.float32)
    # q/k weights: each output channel stored twice (cols j and j+64) so
    # the GEMM emits duplicated q/k for the row-tiled S matmuls; wk/bk
    # pre-scaled so s' = APRIME*SCALE*(k.q) comes out of the PE.
    wqT = np.zeros((CIN + 1, 2 * C), np.float32)
    wqT[:CIN, :C] = np.asarray(wq, np.float32).T
    wqT[CIN, :C] = np.asarray(bq, np.float32)
    wqT[:, C:] = wqT[:, :C]
    wkT = np.zeros((CIN + 1, 2 * C), np.float32)
    wkT[:CIN, :C] = np.asarray(wk, np.float32).T * KSCL
    wkT[CIN, :C] = np.asarray(bk, np.float32) * KSCL
    wkT[:, C:] = wkT[:, :C]
    wvT = np.zeros((CIN + 1, C + 2), np.float32)
    wvT[:CIN, :C] = np.asarray(wv, np.float32).T
    wvT[CIN, :C] = np.asarray(bv, np.float32)
    wvT[CIN, C] = 1.0
    ones = np.ones((1, N), np.float32)
    maps = []
    for i in range(B):
        xa = np.concatenate([x[i].reshape(CIN, N), ones], 0)
        maps.append({"x_aug": np.ascontiguousarray(xa),
                     "wqT_aug": wqT, "wkT_aug": wkT, "wvT_aug": wvT})
    return maps


def kernel(x, wq, bq, wk, bk, wv, bv):
    from concourse.bass_utils import run_bass_kernel_spmd

    if "nc" not in _CACHE:
        _CACHE["nc"] = _build()
    nc = _CACHE["nc"]
    in_maps = _prep_inputs(x, wq, bq, wk, bk, wv, bv)
    res = run_bass_kernel_spmd(nc, in_maps, list(range(B)))
    out = np.stack([res.results[i]["out"].reshape(C, H, W) for i in range(B)])
    return out.astype(np.float32)
